# revision 16
# baseline (speedup 1.0000x reference)
"""BinaryLinear (straight-through sign(w)) kernel for Trainium2, 8 NeuronCores.

Computes out = x @ sign(w).T + b for
  x: [8192, 2048] f32, w: [4096, 2048] f32, b: [4096] f32 -> out [8192, 4096] f32.

Sharding: 4-way data parallel (batch) x 2-way tensor parallel (out_features).
Each core computes a [2048, 2048] block of the output:
  out[bi*2048:(bi+1)*2048, fi*2048:(fi+1)*2048]
    = x_shard @ sign(w_shard).T + b_shard.

Per-core device kernel (fp16 matmul, fp32 accumulate — fp16 runs at the same
PE rate as bf16 but keeps 10 mantissa bits; sign(w) in {-1,0,1} is exact):
  - the whole w^T shard [2048, 2048] fp16 lives in SBUF (64 KiB/partition),
    loaded once;
  - x^T tiles stream through a multi-buffered pool;
  - bias is added during the PSUM->SBUF copyback on the vector engine.
"""

from contextlib import ExitStack

import numpy as np

# Full problem shapes (hardcoded per the grading contract).
M, K, N = 8192, 2048, 4096
P_BATCH, P_FEAT = 4, 2  # 4 x 2 core grid
MC, NC = M // P_BATCH, N // P_FEAT  # 2048, 2048 per-core block
N_CORES = P_BATCH * P_FEAT
P = 128


def build_nc(mc: int = MC, k: int = K, nc_dim: int = NC, reps: int = 1):
    """Build + compile the per-core Bass module: out[mc, nc_dim] = xt^T @ wt + bias.

    reps > 1 repeats the whole computation (for slope-based benchmarking)."""
    import concourse.mybir as mybir
    import concourse.tile as tile
    from concourse import bacc
    from concourse.bass import ts
    from concourse.kernels.tile_matmul import (
        ShapeInfo,
        composable_matmul_tile_kernel,
    )

    ko = k // P
    nc = bacc.Bacc("TRN2", target_bir_lowering=False, debug=False)
    xt = nc.dram_tensor("xt", [k, mc], mybir.dt.float16, kind="ExternalInput")
    wt = nc.dram_tensor("wt", [k, nc_dim], mybir.dt.float16, kind="ExternalInput")
    bias = nc.dram_tensor("bias", [nc_dim], mybir.dt.float32, kind="ExternalInput")
    out = nc.dram_tensor("out", [mc, nc_dim], mybir.dt.float32, kind="ExternalOutput")

    MAX_K_TILE = 512
    k_tile = min(MAX_K_TILE, k)
    k_tiles = k // k_tile
    k_subtiles = k_tile // P

    with tile.TileContext(nc) as tc, ExitStack() as ctx:
        const = ctx.enter_context(tc.tile_pool(name="const", bufs=1))
        kxm_pool = ctx.enter_context(tc.tile_pool(name="kxm", bufs=k_tiles + 1))

        # Whole w^T shard resident in SBUF, laid out [p, ko, n] with
        # cache[p, o, n] = wt[o*128 + p, n]. Preload runs on the gpsimd (SWDGE)
        # queue so the x-tile loads (HWDGE via nc.sync) are not serialized
        # behind it, and in fine chunks ordered so the chunk the first matmuls
        # need lands first.
        w_sb = const.tile([P, ko, nc_dim], mybir.dt.float16)
        wt_t = wt.ap().rearrange("(o p) n -> p o n", p=P)
        # n-major order: the first output tile consumes (n0, k0..k3), so all
        # its chunks must land first. The very first chunk is split per
        # k-subtile so the first matmul unblocks after ~128 KiB.
        n_chunk = max(512, nc_dim // 4)
        for n0 in range(0, nc_dim, n_chunk):
            for kt in range(k_tiles):
                sl = slice(kt * k_subtiles, (kt + 1) * k_subtiles)
                if n0 == 0 and kt == 0:
                    for s in range(k_subtiles):
                        nc.gpsimd.dma_start(
                            out=w_sb[:, s : s + 1, n0 : n0 + n_chunk],
                            in_=wt_t[:, s : s + 1, n0 : n0 + n_chunk],
                        )
                else:
                    nc.gpsimd.dma_start(
                        out=w_sb[:, sl, n0 : n0 + n_chunk],
                        in_=wt_t[:, sl, n0 : n0 + n_chunk],
                    )

        # Bias replicated across all 128 partitions so the copyback can add the
        # n-slice with a plain tensor_tensor add. First needed only at the
        # first PSUM eviction, so it goes after the first w chunks.
        bias_sb = const.tile([P, nc_dim], mybir.dt.float32)
        nc.gpsimd.dma_start(
            out=bias_sb[:], in_=bias.ap()[None, :].to_broadcast((P, nc_dim))
        )

        # Custom kxm producer: one DMA per k-subtile (instead of one per
        # k-tile) so the first matmuls unblock sooner and later tiles
        # prefetch at finer granularity.
        xt_t = xt.ap().rearrange("(o p) m -> p o m", p=P)

        def kxm_producer(nc_, md):
            t = kxm_pool.tile([P, md.k_subtiles, md.m_tile], mybir.dt.float16, tag="kxm")
            m0 = md.m_tile_idx * md.m_tile
            o0 = md.k_tile_idx * md.k_subtiles
            if md.k_tile_idx == 0 and md.m_tile_idx == 0:
                # Fine-grained only on the critical first tile so the first
                # matmul unblocks after one k-subtile instead of four.
                for s in range(md.k_subtiles):
                    nc_.sync.dma_start(
                        out=t[:, s, :], in_=xt_t[:, o0 + s, m0 : m0 + md.m_tile]
                    )
            else:
                nc_.sync.dma_start(
                    out=t[:],
                    in_=xt_t[:, o0 : o0 + md.k_subtiles, m0 : m0 + md.m_tile],
                )
            return t

        kxm_shape = ShapeInfo(pdims=((P, ko),), fdims=(mc,))

        def kxn_producer(nc_, md):
            return w_sb[:, ts(md.k_tile_idx, md.k_subtiles), ts(md.n_tile_idx, md.n_tile)]

        kxn_shape = ShapeInfo(pdims=((P, ko),), fdims=(nc_dim,))

        out_t = out.ap().rearrange("(o p) n -> p o n", p=P)

        def add_bias_store_reducer(nc_, psum, sbuf, md):
            # psum -> sbuf with the bias added, then store this subtile
            # immediately (finer-grained than the stock whole-tile consumer,
            # so stores overlap the remaining evictions and the tail drains
            # faster).
            sz = md.n_subtile_slice_size
            nc_.vector.tensor_add(
                out=sbuf[:, :, :sz],
                in0=psum[:, :sz],
                in1=bias_sb[: psum.shape[0], md.n_subtile_slice],
            )
            po = md.m_tile_idx * md.m_subtiles + md.m_subtile_idx
            nc_.sync.dma_start(
                out=out_t[:, po : po + 1, md.n_subtile_slice], in_=sbuf[:, :, :sz]
            )

        for _ in range(reps):
            composable_matmul_tile_kernel(
                tc=tc,
                kxm_shape=kxm_shape,
                kxn_shape=kxn_shape,
                output_type=mybir.dt.float32,
                kxm_producer=kxm_producer,
                kxn_producer=kxn_producer,
                mxn_consumer=lambda nc_, tile_, md: None,
                mxn_subtile_reducer=add_bias_store_reducer,
                MAX_K_TILE_SIZE=MAX_K_TILE,
                psum_n_bufs=2,
            )

    nc.compile()
    return nc


_NC_CACHE = None


def _get_nc():
    global _NC_CACHE
    if _NC_CACHE is None:
        _NC_CACHE = build_nc()
    return _NC_CACHE


def kernel(x: np.ndarray, w: np.ndarray, b: np.ndarray) -> np.ndarray:
    from concourse.bass_utils import run_bass_kernel_spmd

    x = np.asarray(x, dtype=np.float32)
    w = np.asarray(w, dtype=np.float32)
    b = np.asarray(b, dtype=np.float32)

    f16 = np.float16
    x_f16 = x.astype(f16)
    w_f16 = np.sign(w).astype(f16)

    # Unique transposed shards (x^T per batch group, sign(w)^T per feature
    # group), transposed in parallel (numpy releases the GIL on these copies).
    from concurrent.futures import ThreadPoolExecutor

    with ThreadPoolExecutor(max_workers=6) as pool:
        xt_f = [
            pool.submit(np.ascontiguousarray, x_f16[bi * MC : (bi + 1) * MC, :].T)
            for bi in range(P_BATCH)
        ]
        wt_f = [
            pool.submit(np.ascontiguousarray, w_f16[fi * NC : (fi + 1) * NC, :].T)
            for fi in range(P_FEAT)
        ]
        xt_shards = [f.result() for f in xt_f]
        wt_shards = [f.result() for f in wt_f]
    b_shards = [np.ascontiguousarray(b[fi * NC : (fi + 1) * NC]) for fi in range(P_FEAT)]

    in_maps = []
    for c in range(N_CORES):
        bi, fi = divmod(c, P_FEAT)
        in_maps.append(
            {"xt": xt_shards[bi], "wt": wt_shards[fi], "bias": b_shards[fi]}
        )

    nc = _get_nc()
    try:
        results = run_bass_kernel_spmd(
            nc, in_maps, core_ids=list(range(N_CORES))
        ).results
    except Exception:
        # One retry for transient runtime/relay failures.
        results = run_bass_kernel_spmd(
            nc, in_maps, core_ids=list(range(N_CORES))
        ).results

    out = np.empty((M, N), dtype=np.float32)
    for c in range(N_CORES):
        bi, fi = divmod(c, P_FEAT)
        out[bi * MC : (bi + 1) * MC, fi * NC : (fi + 1) * NC] = results[c]["out"]
    return out


# revision 21
# speedup vs baseline: 1.0015x; 1.0015x over previous
"""BinaryLinear (straight-through sign(w)) kernel for Trainium2, 8 NeuronCores.

Computes out = x @ sign(w).T + b for
  x: [8192, 2048] f32, w: [4096, 2048] f32, b: [4096] f32 -> out [8192, 4096] f32.

Sharding: 4-way data parallel (batch) x 2-way tensor parallel (out_features).
Each core computes a [2048, 2048] block of the output:
  out[bi*2048:(bi+1)*2048, fi*2048:(fi+1)*2048]
    = x_shard @ sign(w_shard).T + b_shard.

Per-core device kernel (fp16 matmul, fp32 accumulate — fp16 runs at the same
PE rate as bf16 but keeps 10 mantissa bits; sign(w) in {-1,0,1} is exact):
  - the whole w^T shard [2048, 2048] fp16 lives in SBUF (64 KiB/partition),
    loaded once;
  - x^T tiles stream through a multi-buffered pool;
  - bias is added during the PSUM->SBUF copyback on the vector engine.
"""

from contextlib import ExitStack

import numpy as np

# Full problem shapes (hardcoded per the grading contract).
M, K, N = 8192, 2048, 4096
P_BATCH, P_FEAT = 4, 2  # 4 x 2 core grid
MC, NC = M // P_BATCH, N // P_FEAT  # 2048, 2048 per-core block
N_CORES = P_BATCH * P_FEAT
P = 128


def build_nc(mc: int = MC, k: int = K, nc_dim: int = NC, reps: int = 1):
    """Build + compile the per-core Bass module: out[mc, nc_dim] = xt^T @ wt + bias.

    reps > 1 repeats the whole computation (for slope-based benchmarking)."""
    import concourse.mybir as mybir
    import concourse.tile as tile
    from concourse import bacc
    from concourse.bass import ts
    from concourse.kernels.tile_matmul import (
        ShapeInfo,
        composable_matmul_tile_kernel,
    )

    ko = k // P
    MAX_K_TILE = 512
    k_tile = min(MAX_K_TILE, k)
    k_tiles = k // k_tile
    k_subtiles = k_tile // P
    TB = 512  # m/n tile width of the pre-blocked host layouts
    m_tiles = mc // TB
    n_blocks = nc_dim // TB

    nc = bacc.Bacc("TRN2", target_bir_lowering=False, debug=False)
    # Inputs arrive pre-blocked on the host (see _pack_kxm/_pack_kxn): each
    # [P, k_subtiles, TB] block is fully contiguous in DRAM, so every DMA has
    # 4-KiB-per-partition descriptor runs instead of 1-KiB strided ones.
    xt = nc.dram_tensor(
        "xt", [m_tiles, k_tiles, P, k_subtiles, TB], mybir.dt.float16,
        kind="ExternalInput",
    )
    wt = nc.dram_tensor(
        "wt", [n_blocks, k_tiles, P, k_subtiles, TB], mybir.dt.float16,
        kind="ExternalInput",
    )
    bias = nc.dram_tensor("bias", [nc_dim], mybir.dt.float32, kind="ExternalInput")
    out = nc.dram_tensor("out", [mc, nc_dim], mybir.dt.float32, kind="ExternalOutput")

    with tile.TileContext(nc) as tc, ExitStack() as ctx:
        const = ctx.enter_context(tc.tile_pool(name="const", bufs=1))
        kxm_pool = ctx.enter_context(tc.tile_pool(name="kxm", bufs=k_tiles + 1))

        # Whole w^T shard resident in SBUF, laid out [p, ko, n] with
        # cache[p, o, n] = w^T[o*128 + p, n]. Preload runs on the gpsimd
        # (SWDGE) queue so the x-tile loads (HWDGE via nc.sync) are not
        # serialized behind it, in n-major order: the first output tile
        # consumes (n0, k0..k3), so all its chunks must land first.
        w_sb = const.tile([P, ko, nc_dim], mybir.dt.float16)
        for nb in range(n_blocks):
            for kt in range(k_tiles):
                sl = slice(kt * k_subtiles, (kt + 1) * k_subtiles)
                nc.gpsimd.dma_start(
                    out=w_sb[:, sl, nb * TB : (nb + 1) * TB],
                    in_=wt.ap()[nb, kt],
                )

        # Bias replicated across all 128 partitions so the copyback can add the
        # n-slice with a plain tensor_tensor add. First needed only at the
        # first PSUM eviction, so it goes after the first w chunks.
        bias_sb = const.tile([P, nc_dim], mybir.dt.float32)
        nc.gpsimd.dma_start(
            out=bias_sb[:], in_=bias.ap()[None, :].to_broadcast((P, nc_dim))
        )

        # Custom kxm producer: one contiguous-block DMA per k-tile of x^T.
        def kxm_producer(nc_, md):
            t = kxm_pool.tile([P, md.k_subtiles, md.m_tile], mybir.dt.float16, tag="kxm")
            nc_.sync.dma_start(out=t[:], in_=xt.ap()[md.m_tile_idx, md.k_tile_idx])
            return t

        kxm_shape = ShapeInfo(pdims=((P, ko),), fdims=(mc,))

        def kxn_producer(nc_, md):
            return w_sb[:, ts(md.k_tile_idx, md.k_subtiles), ts(md.n_tile_idx, md.n_tile)]

        kxn_shape = ShapeInfo(pdims=((P, ko),), fdims=(nc_dim,))

        out_t = out.ap().rearrange("(o p) n -> p o n", p=P)

        def add_bias_store_reducer(nc_, psum, sbuf, md):
            # psum -> sbuf with the bias added, then store this subtile
            # immediately (finer-grained than the stock whole-tile consumer,
            # so stores overlap the remaining evictions and the tail drains
            # faster).
            sz = md.n_subtile_slice_size
            nc_.vector.tensor_add(
                out=sbuf[:, :, :sz],
                in0=psum[:, :sz],
                in1=bias_sb[: psum.shape[0], md.n_subtile_slice],
            )
            po = md.m_tile_idx * md.m_subtiles + md.m_subtile_idx
            nc_.sync.dma_start(
                out=out_t[:, po : po + 1, md.n_subtile_slice], in_=sbuf[:, :, :sz]
            )

        for _ in range(reps):
            composable_matmul_tile_kernel(
                tc=tc,
                kxm_shape=kxm_shape,
                kxn_shape=kxn_shape,
                output_type=mybir.dt.float32,
                kxm_producer=kxm_producer,
                kxn_producer=kxn_producer,
                mxn_consumer=lambda nc_, tile_, md: None,
                mxn_subtile_reducer=add_bias_store_reducer,
                MAX_K_TILE_SIZE=MAX_K_TILE,
                psum_n_bufs=2,
            )

    nc.compile()
    return nc


def _pack_blocks(a: np.ndarray, tb: int = 512) -> np.ndarray:
    """[F, K] row-major -> [F//tb, K//ktw, 128, ks, tb] DMA-contiguous blocks.

    block[ft, kt, p, s, j] = a[ft*tb + j, kt*ktw + s*128 + p], i.e. each
    [128, ks, tb] block is one fully-contiguous DMA source with K on the
    partition dim (a^T layout within the block)."""
    f, k = a.shape
    ktw = min(512, k)
    kts, ks = k // ktw, ktw // P
    v = a.reshape(f // tb, tb, kts, ks, P)
    return np.ascontiguousarray(v.transpose(0, 2, 4, 3, 1))


_NC_CACHE = None


def _get_nc():
    global _NC_CACHE
    if _NC_CACHE is None:
        _NC_CACHE = build_nc()
    return _NC_CACHE


def kernel(x: np.ndarray, w: np.ndarray, b: np.ndarray) -> np.ndarray:
    from concourse.bass_utils import run_bass_kernel_spmd

    x = np.asarray(x, dtype=np.float32)
    w = np.asarray(w, dtype=np.float32)
    b = np.asarray(b, dtype=np.float32)

    f16 = np.float16
    x_f16 = x.astype(f16)
    w_f16 = np.sign(w).astype(f16)

    # Unique DMA-blocked shards (x per batch group, sign(w) per feature
    # group), packed in parallel (numpy releases the GIL on these copies).
    from concurrent.futures import ThreadPoolExecutor

    with ThreadPoolExecutor(max_workers=6) as pool:
        xt_f = [
            pool.submit(_pack_blocks, x_f16[bi * MC : (bi + 1) * MC])
            for bi in range(P_BATCH)
        ]
        wt_f = [
            pool.submit(_pack_blocks, w_f16[fi * NC : (fi + 1) * NC])
            for fi in range(P_FEAT)
        ]
        xt_shards = [f.result() for f in xt_f]
        wt_shards = [f.result() for f in wt_f]
    b_shards = [np.ascontiguousarray(b[fi * NC : (fi + 1) * NC]) for fi in range(P_FEAT)]

    in_maps = []
    for c in range(N_CORES):
        bi, fi = divmod(c, P_FEAT)
        in_maps.append(
            {"xt": xt_shards[bi], "wt": wt_shards[fi], "bias": b_shards[fi]}
        )

    nc = _get_nc()
    try:
        results = run_bass_kernel_spmd(
            nc, in_maps, core_ids=list(range(N_CORES))
        ).results
    except Exception:
        # One retry for transient runtime/relay failures.
        results = run_bass_kernel_spmd(
            nc, in_maps, core_ids=list(range(N_CORES))
        ).results

    out = np.empty((M, N), dtype=np.float32)
    for c in range(N_CORES):
        bi, fi = divmod(c, P_FEAT)
        out[bi * MC : (bi + 1) * MC, fi * NC : (fi + 1) * NC] = results[c]["out"]
    return out


# revision 24
# speedup vs baseline: 1.0088x; 1.0072x over previous
"""BinaryLinear (straight-through sign(w)) kernel for Trainium2, 8 NeuronCores.

Computes out = x @ sign(w).T + b for
  x: [8192, 2048] f32, w: [4096, 2048] f32, b: [4096] f32 -> out [8192, 4096] f32.

Sharding: 4-way data parallel (batch) x 2-way tensor parallel (out_features).
Each core computes a [2048, 2048] block of the output:
  out[bi*2048:(bi+1)*2048, fi*2048:(fi+1)*2048]
    = x_shard @ sign(w_shard).T + b_shard.

Per-core device kernel (fp16 matmul, fp32 accumulate — fp16 runs at the same
PE rate as bf16 but keeps 10 mantissa bits; sign(w) in {-1,0,1} is exact):
  - the whole w^T shard [2048, 2048] fp16 lives in SBUF (64 KiB/partition),
    loaded once;
  - x^T tiles stream through a multi-buffered pool;
  - bias is added during the PSUM->SBUF copyback on the vector engine.
"""

from contextlib import ExitStack

import numpy as np

# Full problem shapes (hardcoded per the grading contract).
M, K, N = 8192, 2048, 4096
P_BATCH, P_FEAT = 4, 2  # 4 x 2 core grid
MC, NC = M // P_BATCH, N // P_FEAT  # 2048, 2048 per-core block
N_CORES = P_BATCH * P_FEAT
P = 128


def build_nc(mc: int = MC, k: int = K, nc_dim: int = NC, reps: int = 1):
    """Build + compile the per-core Bass module: out[mc, nc_dim] = xt^T @ wt + bias.

    reps > 1 repeats the whole computation (for slope-based benchmarking)."""
    import concourse.mybir as mybir
    import concourse.tile as tile
    from concourse import bacc
    from concourse.bass import ts
    from concourse.kernels.tile_matmul import (
        ShapeInfo,
        composable_matmul_tile_kernel,
    )

    ko = k // P
    MAX_K_TILE = 512
    k_tile = min(MAX_K_TILE, k)
    k_tiles = k // k_tile
    k_subtiles = k_tile // P
    TB = 512  # m/n tile width of the pre-blocked host layouts
    m_tiles = mc // TB
    n_blocks = nc_dim // TB

    nc = bacc.Bacc("TRN2", target_bir_lowering=False, debug=False)
    # Inputs arrive pre-blocked on the host (see _pack_blocks): each
    # [P, k_subtiles, TB] block is fully contiguous in DRAM, so every DMA has
    # 4-KiB-per-partition descriptor runs instead of 1-KiB strided ones.
    xt = nc.dram_tensor(
        "xt", [m_tiles, k_tiles, P, k_subtiles, TB], mybir.dt.float16,
        kind="ExternalInput",
    )
    wt = nc.dram_tensor(
        "wt", [n_blocks, k_tiles, P, k_subtiles, TB], mybir.dt.float16,
        kind="ExternalInput",
    )
    bias = nc.dram_tensor("bias", [nc_dim], mybir.dt.float32, kind="ExternalInput")
    out = nc.dram_tensor("out", [mc, nc_dim], mybir.dt.float32, kind="ExternalOutput")

    with tile.TileContext(nc) as tc, ExitStack() as ctx:
        # HAM warmup: the PE clock is gated to 1.2 GHz until ~3.4 us of
        # sustained activity. The first real matmuls can't start until their
        # operands arrive (~5 us of DMA ramp), so spend the idle window on
        # throwaway matmuls over a zeroed scratch tile — the cold-clock
        # penalty lands on them instead of the real work. The scratch SBUF
        # pool stays OPEN so its slot is never reused (a close would order
        # the w preload behind the dummy reads); only the PSUM bank is
        # returned before the real kernel needs all 8.
        warm_sb = ctx.enter_context(tc.tile_pool(name="warm_sb", bufs=1))
        scratch = warm_sb.tile([P, 512], mybir.dt.float16)
        nc.vector.memset(scratch[:], 0.0)
        with tc.tile_pool(name="warm_ps", bufs=1, space="PSUM") as wps_pool:
            ps = wps_pool.tile([P, 512], mybir.dt.float32)
            for _ in range(10):
                nc.tensor.matmul(
                    ps[:], scratch[:, :P], scratch[:], start=True, stop=True
                )

        const = ctx.enter_context(tc.tile_pool(name="const", bufs=1))
        kxm_pool = ctx.enter_context(tc.tile_pool(name="kxm", bufs=k_tiles + 1))

        # Whole w^T shard resident in SBUF, laid out [p, ko, n] with
        # cache[p, o, n] = w^T[o*128 + p, n]. Preload runs on the gpsimd
        # (SWDGE) queue so the x-tile loads (HWDGE via nc.sync) are not
        # serialized behind it, in n-major order: the first output tile
        # consumes (n0, k0..k3), so all its chunks must land first.
        w_sb = const.tile([P, ko, nc_dim], mybir.dt.float16)
        for nb in range(n_blocks):
            for kt in range(k_tiles):
                sl = slice(kt * k_subtiles, (kt + 1) * k_subtiles)
                nc.gpsimd.dma_start(
                    out=w_sb[:, sl, nb * TB : (nb + 1) * TB],
                    in_=wt.ap()[nb, kt],
                )

        # Bias replicated across all 128 partitions so the copyback can add the
        # n-slice with a plain tensor_tensor add. One tiny [1, N] HBM read on
        # the otherwise-idle ACT HWDGE ring + an on-chip partition broadcast —
        # a [128, N] broadcast DMA on the SWDGE queue behind the w preload
        # would block the first evictions (and PSUM recycling) until ~36 us.
        bias_sb = const.tile([P, nc_dim], mybir.dt.float32)
        nc.scalar.dma_start(out=bias_sb[:1, :], in_=bias.ap()[None, :])
        nc.gpsimd.partition_broadcast(bias_sb[:], bias_sb[:1, :])

        # Custom kxm producer: one contiguous-block DMA per k-tile of x^T.
        def kxm_producer(nc_, md):
            t = kxm_pool.tile([P, md.k_subtiles, md.m_tile], mybir.dt.float16, tag="kxm")
            nc_.sync.dma_start(out=t[:], in_=xt.ap()[md.m_tile_idx, md.k_tile_idx])
            return t

        kxm_shape = ShapeInfo(pdims=((P, ko),), fdims=(mc,))

        def kxn_producer(nc_, md):
            return w_sb[:, ts(md.k_tile_idx, md.k_subtiles), ts(md.n_tile_idx, md.n_tile)]

        kxn_shape = ShapeInfo(pdims=((P, ko),), fdims=(nc_dim,))

        out_t = out.ap().rearrange("(o p) n -> p o n", p=P)

        def add_bias_store_reducer(nc_, psum, sbuf, md):
            # psum -> sbuf with the bias added, then store this subtile
            # immediately (finer-grained than the stock whole-tile consumer,
            # so stores overlap the remaining evictions and the tail drains
            # faster).
            sz = md.n_subtile_slice_size
            nc_.vector.tensor_add(
                out=sbuf[:, :, :sz],
                in0=psum[:, :sz],
                in1=bias_sb[: psum.shape[0], md.n_subtile_slice],
            )
            po = md.m_tile_idx * md.m_subtiles + md.m_subtile_idx
            nc_.sync.dma_start(
                out=out_t[:, po : po + 1, md.n_subtile_slice], in_=sbuf[:, :, :sz]
            )

        for _ in range(reps):
            composable_matmul_tile_kernel(
                tc=tc,
                kxm_shape=kxm_shape,
                kxn_shape=kxn_shape,
                output_type=mybir.dt.float32,
                kxm_producer=kxm_producer,
                kxn_producer=kxn_producer,
                mxn_consumer=lambda nc_, tile_, md: None,
                mxn_subtile_reducer=add_bias_store_reducer,
                MAX_K_TILE_SIZE=MAX_K_TILE,
                psum_n_bufs=2,
            )

    nc.compile()
    return nc


def _pack_blocks(a: np.ndarray, tb: int = 512) -> np.ndarray:
    """[F, K] row-major -> [F//tb, K//ktw, 128, ks, tb] DMA-contiguous blocks.

    block[ft, kt, p, s, j] = a[ft*tb + j, kt*ktw + s*128 + p], i.e. each
    [128, ks, tb] block is one fully-contiguous DMA source with K on the
    partition dim (a^T layout within the block)."""
    f, k = a.shape
    ktw = min(512, k)
    kts, ks = k // ktw, ktw // P
    v = a.reshape(f // tb, tb, kts, ks, P)
    return np.ascontiguousarray(v.transpose(0, 2, 4, 3, 1))


_NC_CACHE = None


def _get_nc():
    global _NC_CACHE
    if _NC_CACHE is None:
        _NC_CACHE = build_nc()
    return _NC_CACHE


def kernel(x: np.ndarray, w: np.ndarray, b: np.ndarray) -> np.ndarray:
    from concourse.bass_utils import run_bass_kernel_spmd

    x = np.asarray(x, dtype=np.float32)
    w = np.asarray(w, dtype=np.float32)
    b = np.asarray(b, dtype=np.float32)

    f16 = np.float16
    x_f16 = x.astype(f16)
    w_f16 = np.sign(w).astype(f16)

    # Unique DMA-blocked shards (x per batch group, sign(w) per feature
    # group), packed in parallel (numpy releases the GIL on these copies).
    from concurrent.futures import ThreadPoolExecutor

    with ThreadPoolExecutor(max_workers=6) as pool:
        xt_f = [
            pool.submit(_pack_blocks, x_f16[bi * MC : (bi + 1) * MC])
            for bi in range(P_BATCH)
        ]
        wt_f = [
            pool.submit(_pack_blocks, w_f16[fi * NC : (fi + 1) * NC])
            for fi in range(P_FEAT)
        ]
        xt_shards = [f.result() for f in xt_f]
        wt_shards = [f.result() for f in wt_f]
    b_shards = [np.ascontiguousarray(b[fi * NC : (fi + 1) * NC]) for fi in range(P_FEAT)]

    in_maps = []
    for c in range(N_CORES):
        bi, fi = divmod(c, P_FEAT)
        in_maps.append(
            {"xt": xt_shards[bi], "wt": wt_shards[fi], "bias": b_shards[fi]}
        )

    nc = _get_nc()
    try:
        results = run_bass_kernel_spmd(
            nc, in_maps, core_ids=list(range(N_CORES))
        ).results
    except Exception:
        # One retry for transient runtime/relay failures.
        results = run_bass_kernel_spmd(
            nc, in_maps, core_ids=list(range(N_CORES))
        ).results

    out = np.empty((M, N), dtype=np.float32)
    for c in range(N_CORES):
        bi, fi = divmod(c, P_FEAT)
        out[bi * MC : (bi + 1) * MC, fi * NC : (fi + 1) * NC] = results[c]["out"]
    return out
